# revision 3
# baseline (speedup 1.0000x reference)
"""Trainium2 Bass kernel for a single causal attention head.

Problem: x [8, 2048, 1024] f32, Wq/Wk/Wv [1024, 64] f32.
out[b] = softmax(causal(x[b] Wq (x[b] Wk)^T) / 8) @ (x[b] Wv)   -> [8, 2048, 64] f32

Sharding: data-parallel over batch. Each of the 8 NeuronCores runs the same
single-core program on its own batch element (no collectives).

Per-core dataflow (all matmuls in bf16, fp32 PSUM accumulation):
  1. SWDGE cast-DMA x f32->bf16 into SBUF, then xbar DMA-transpose to get
     x^T (d on partitions) which every d-contraction needs.
  2. Projections: Q^T/K^T/V^T [64, S] = W[dchunk].T @ x^T[dchunk] accumulated
     over d-chunks.  V natural [k,64] is recovered with a matmul-by-identity.
  3. scoresT[k,q] = K^T[:,ktile].T @ Q^T (contraction over h=64), exp on
     ScalarE with the 1/sqrt(64) scale folded in, causal handling by loop
     bounds + memset + one triangular-mask multiply per diagonal block.
  4. attnT @ V via lhsT = [V | ones] so PSUM row 64 accumulates the softmax
     denominators together with the 64 output dims: O^T[65, q].
  5. O = (O^T).T via matmul-by-identity, divide rows by the sums
     (DVE reciprocal + tensor_scalar), DMA out f32.
"""

import math
import os
import sys

import numpy as np

if "/opt/trn_rl_repo" not in sys.path:
    sys.path.insert(0, "/opt/trn_rl_repo")

import concourse.bacc as bacc
import concourse.tile as tile
from concourse import mybir
from concourse.masks import make_identity

BATCH = 8
SEQ = 2048
D_EMBED = 1024
HEAD = 64
N_CORES = 8

F32 = mybir.dt.float32
BF16 = mybir.dt.bfloat16


def build_attention_nc(S=SEQ, D=D_EMBED, repeat=1):
    """Build the single-core Bass program for one batch element.

    repeat > 1 re-emits the whole body (for timing: the delta between
    repeat=R and repeat=1 wall time isolates per-iteration HW time).
    """
    H = HEAD
    ST = S // 128          # s-tiles == k-tiles
    DC = D // 128          # d-chunks
    QW = 512               # q-chunk width
    NQ = S // QW           # q-chunks
    KPQ = QW // 128        # k-tiles per q-chunk
    inv_sqrt_h = 1.0 / math.sqrt(H)

    nc = bacc.Bacc("TRN2", target_bir_lowering=False, debug=False)

    x_dram = nc.dram_tensor("x", [S, D], F32, kind="ExternalInput").ap()
    wq_dram = nc.dram_tensor("Wq", [D, H], F32, kind="ExternalInput").ap()
    wk_dram = nc.dram_tensor("Wk", [D, H], F32, kind="ExternalInput").ap()
    wv_dram = nc.dram_tensor("Wv", [D, H], F32, kind="ExternalInput").ap()
    out_dram = nc.dram_tensor("out", [S, H], F32, kind="ExternalOutput").ap()

    with tile.TileContext(nc) as tc:
      for _rep in range(repeat):
        with tc.tile_pool(name="const", bufs=1) as const:
            # identities for the matmul-by-identity transposes
            ident_h = const.tile([H, H], BF16)
            make_identity(nc, ident_h)
            ident_o = const.tile([H + 1, H + 1], F32)
            make_identity(nc, ident_o)
            # triangular mask for diagonal blocks of attnT:
            # trimask[k_local, q_local] = 1.0 if q_local >= k_local else 0.0
            trimask = const.tile([128, 128], BF16)
            nc.gpsimd.memset(trimask, 1.0)
            nc.gpsimd.affine_select(
                out=trimask,
                in_=trimask,
                compare_op=mybir.AluOpType.is_ge,
                fill=0.0,
                base=0,
                pattern=[[1, 128]],
                channel_multiplier=-1,
            )

            # weights -> bf16 [128, DC, H]; w_sb[p, j, h] = W[j*128+p, h]
            w_sb = []
            for name, wd in (("wq", wq_dram), ("wk", wk_dram), ("wv", wv_dram)):
                w_t = const.tile([128, DC, H], BF16, name=f"w_{name}", tag=f"w_{name}")
                nc.gpsimd.dma_start(out=w_t, in_=wd.rearrange("(j p) h -> p j h", p=128))
                w_sb.append(w_t)
            wq_sb, wk_sb, wv_sb = w_sb

            # persistent activations
            qt_sb = const.tile([64, S], BF16)    # Q^T
            ktr_sb = const.tile([64, S], BF16)   # K^T
            vt_sb = const.tile([64, S], BF16)    # V^T
            vnat_sb = const.tile([128, ST, H + 1], BF16)  # [V | ones] per k-tile
            osb = const.tile([H + 1, S], F32)    # O^T staging (SBUF)
            o_out = const.tile([128, ST, H], F32)  # final output staging

            # ---------- load x (cast f32->bf16) and build x^T ----------
            with tc.tile_pool(name="xload", bufs=1) as xload:
                x_bf = xload.tile([128, ST, D], BF16)
                x_src = x_dram.rearrange("(a p) d -> p a d", p=128)
                n_load = 4 if ST % 4 == 0 else 1
                step = ST // n_load
                for c in range(n_load):
                    nc.gpsimd.dma_start(
                        out=x_bf[:, c * step:(c + 1) * step, :],
                        in_=x_src[:, c * step:(c + 1) * step, :],
                    )

                # xt[p, j, s] = x[s, j*128 + p]  (== x^T[j*128+p, s])
                xt = xload.tile([128, DC, S], BF16)
                for si in range(ST):
                    nc.sync.dma_start(
                        out=xt[:, :, si * 128:(si + 1) * 128],
                        in_=x_bf[:, si, :],
                        transpose=True,
                    )

                # ---------- projections ----------
                with tc.tile_pool(name="proj_ps", bufs=2, space="PSUM") as proj_ps:
                    for w_t, dst in ((wq_sb, qt_sb), (wk_sb, ktr_sb), (wv_sb, vt_sb)):
                        for qc in range(NQ):
                            pp = proj_ps.tile([64, QW], F32)
                            for j in range(DC):
                                nc.tensor.matmul(
                                    pp,
                                    lhsT=w_t[:, j, :],
                                    rhs=xt[:, j, qc * QW:(qc + 1) * QW],
                                    start=(j == 0),
                                    stop=(j == DC - 1),
                                )
                            nc.vector.tensor_copy(dst[:, qc * QW:(qc + 1) * QW], pp)

                # ---------- V natural (+ ones column) ----------
                with tc.tile_pool(name="v_ps", bufs=2, space="PSUM") as v_ps:
                    for st in range(ST):
                        vp = v_ps.tile([128, H], F32)
                        nc.tensor.matmul(
                            vp,
                            lhsT=vt_sb[:, st * 128:(st + 1) * 128],
                            rhs=ident_h,
                            start=True,
                            stop=True,
                        )
                        nc.scalar.copy(vnat_sb[:, st, 0:H], vp)
                nc.vector.memset(vnat_sb[:, :, H:H + 1], 1.0)

            # ---------- main attention loop over k-tiles ----------
            with (
                tc.tile_pool(name="attn_pool", bufs=3) as attn_pool,
                tc.tile_pool(name="s_ps", bufs=2, space="PSUM") as s_ps,
                tc.tile_pool(name="o_ps", bufs=1, space="PSUM") as o_ps,
            ):
                opsum = [
                    o_ps.tile([H + 1, QW], F32, name=f"opsum_{qc}", tag=f"opsum_{qc}")
                    for qc in range(NQ)
                ]
                for kt in range(ST):
                    qc0 = (kt * 128) // QW
                    m = kt - qc0 * KPQ  # k-tile position within its q-chunk
                    attn = attn_pool.tile([128, S], BF16)
                    for qc in range(qc0, NQ):
                        sp = s_ps.tile([128, QW], F32)
                        nc.tensor.matmul(
                            sp,
                            lhsT=ktr_sb[:, kt * 128:(kt + 1) * 128],
                            rhs=qt_sb[:, qc * QW:(qc + 1) * QW],
                            start=True,
                            stop=True,
                        )
                        lo = m * 128 if qc == qc0 else 0
                        nc.scalar.activation(
                            out=attn[:, qc * QW + lo:(qc + 1) * QW],
                            in_=sp[:, lo:QW],
                            func=mybir.ActivationFunctionType.Exp,
                            scale=inv_sqrt_h,
                        )
                    if m > 0:
                        nc.vector.memset(attn[:, qc0 * QW:qc0 * QW + m * 128], 0.0)
                    # triangular mask on the diagonal block
                    nc.vector.tensor_mul(
                        attn[:, kt * 128:(kt + 1) * 128],
                        attn[:, kt * 128:(kt + 1) * 128],
                        trimask,
                    )
                    for qc in range(qc0, NQ):
                        nc.tensor.matmul(
                            opsum[qc],
                            lhsT=vnat_sb[:, kt, :],
                            rhs=attn[:, qc * QW:(qc + 1) * QW],
                            start=(kt == 0),
                            stop=(kt == (qc + 1) * KPQ - 1),
                        )

                # ---------- finalize: transpose O^T -> O, normalize ----------
                for qc in range(NQ):
                    nc.vector.tensor_copy(osb[:, qc * QW:(qc + 1) * QW], opsum[qc])

            with (
                tc.tile_pool(name="fin_ps", bufs=2, space="PSUM") as fin_ps,
                tc.tile_pool(name="fin_sb", bufs=2) as fin_sb,
            ):
                for t in range(ST):
                    op = fin_ps.tile([128, H + 1], F32)
                    nc.tensor.matmul(
                        op,
                        lhsT=osb[:, t * 128:(t + 1) * 128],
                        rhs=ident_o,
                        start=True,
                        stop=True,
                    )
                    recip = fin_sb.tile([128, 1], F32)
                    nc.vector.reciprocal(recip, op[:, H:H + 1])
                    nc.vector.tensor_scalar_mul(o_out[:, t, :], op[:, 0:H], recip)

            nc.sync.dma_start(
                out=out_dram.rearrange("(t p) h -> p t h", p=128),
                in_=o_out,
            )

    nc.compile()
    return nc


_NC_CACHE = {}


def _get_nc(S=SEQ, D=D_EMBED):
    key = (S, D)
    if key not in _NC_CACHE:
        _NC_CACHE[key] = build_attention_nc(S, D)
    return _NC_CACHE[key]


def kernel(x, Wq, Wk, Wv):
    """Full-input entry point: x [8, 2048, 1024] f32 -> [8, 2048, 64] f32."""
    from concourse.bass_utils import run_bass_kernel_spmd

    x = np.asarray(x, dtype=np.float32)
    Wq = np.ascontiguousarray(np.asarray(Wq, dtype=np.float32))
    Wk = np.ascontiguousarray(np.asarray(Wk, dtype=np.float32))
    Wv = np.ascontiguousarray(np.asarray(Wv, dtype=np.float32))
    assert x.shape == (BATCH, SEQ, D_EMBED), x.shape

    nc = _get_nc()
    in_maps = [
        {"x": np.ascontiguousarray(x[b]), "Wq": Wq, "Wk": Wk, "Wv": Wv}
        for b in range(BATCH)
    ]
    res = run_bass_kernel_spmd(nc, in_maps, core_ids=list(range(N_CORES)))
    return np.stack([res.results[b]["out"] for b in range(BATCH)], axis=0)


# revision 4
# speedup vs baseline: 232.7950x; 232.7950x over previous
"""Trainium2 Bass kernel for a single causal attention head.

Problem: x [8, 2048, 1024] f32, Wq/Wk/Wv [1024, 64] f32.
out[b] = softmax(causal(x[b] Wq (x[b] Wk)^T) / 8) @ (x[b] Wv)   -> [8, 2048, 64] f32

Sharding: data-parallel over batch. Each of the 8 NeuronCores runs the same
single-core program on its own batch element (no collectives).

Per-core dataflow (all matmuls in bf16, fp32 PSUM accumulation):
  1. SWDGE cast-DMA x f32->bf16 into SBUF, then xbar DMA-transpose to get
     x^T (d on partitions) which every d-contraction needs.
  2. Projections: col-packed matmuls produce qk_a = [Q^T; K^T] stacked on
     the 128 partitions; a partition-swap SBUF->SBUF DMA builds
     qk_b = [K^T; Q^T] so scores can be row-packed (two k-tiles concurrently
     in the 128x128 PE array, contraction h=64 each).  V^T solo; V natural
     [k,64] recovered with a matmul-by-identity.
  3. scoresT[k,q] = K^T[:,ktile].T @ Q^T, exp on ScalarE (1/sqrt(64) folded
     in), causal handling by loop bounds + memset + one triangular-mask
     multiply per diagonal block.
  4. attnT @ V via lhsT = [V | ones] so PSUM row 64 accumulates the softmax
     denominators together with the 64 output dims: O^T[65, q].
  5. O = (O^T).T via matmul-by-identity, divide rows by the sums
     (DVE reciprocal + tensor_scalar), DMA out f32.
"""

import math
import sys

import numpy as np

if "/opt/trn_rl_repo" not in sys.path:
    sys.path.insert(0, "/opt/trn_rl_repo")

import concourse.bacc as bacc
import concourse.tile as tile
from concourse import mybir
from concourse.masks import make_identity

BATCH = 8
SEQ = 2048
D_EMBED = 1024
HEAD = 64
N_CORES = 8

F32 = mybir.dt.float32
BF16 = mybir.dt.bfloat16


def build_attention_nc(S=SEQ, D=D_EMBED, repeat=1):
    """Build the single-core Bass program for one batch element.

    repeat > 1 re-emits the whole body (for timing: the delta between
    repeat=R and repeat=1 wall time isolates per-iteration HW time).
    """
    H = HEAD
    ST = S // 128          # s-tiles == k-tiles
    DC = D // 128          # d-chunks
    QW = 512               # q-chunk width
    NQ = S // QW           # q-chunks
    KPQ = QW // 128        # k-tiles per q-chunk
    inv_sqrt_h = 1.0 / math.sqrt(H)

    nc = bacc.Bacc("TRN2", target_bir_lowering=False, debug=False)

    x_dram = nc.dram_tensor("x", [S, D], F32, kind="ExternalInput").ap()
    wq_dram = nc.dram_tensor("Wq", [D, H], F32, kind="ExternalInput").ap()
    wk_dram = nc.dram_tensor("Wk", [D, H], F32, kind="ExternalInput").ap()
    wv_dram = nc.dram_tensor("Wv", [D, H], F32, kind="ExternalInput").ap()
    out_dram = nc.dram_tensor("out", [S, H], F32, kind="ExternalOutput").ap()

    with tile.TileContext(nc) as tc:
      for _rep in range(repeat):
        with tc.tile_pool(name="const", bufs=1) as const:
            # identities for the matmul-by-identity transposes
            ident_h = const.tile([H, H], BF16)
            make_identity(nc, ident_h)
            ident_o = const.tile([H + 1, H + 1], F32)
            make_identity(nc, ident_o)
            # triangular mask for diagonal blocks of attnT:
            # trimask[k_local, q_local] = 1.0 if q_local >= k_local else 0.0
            trimask = const.tile([128, 128], BF16)
            nc.gpsimd.memset(trimask, 1.0)
            nc.gpsimd.affine_select(
                out=trimask,
                in_=trimask,
                compare_op=mybir.AluOpType.is_ge,
                fill=0.0,
                base=0,
                pattern=[[1, 128]],
                channel_multiplier=-1,
            )

            # weights -> bf16 [128, DC, H]; w_sb[p, j, h] = W[j*128+p, h]
            w_sb = []
            for name, wd in (("wq", wq_dram), ("wk", wk_dram), ("wv", wv_dram)):
                w_t = const.tile([128, DC, H], BF16, name=f"w_{name}", tag=f"w_{name}")
                nc.gpsimd.dma_start(out=w_t, in_=wd.rearrange("(j p) h -> p j h", p=128))
                w_sb.append(w_t)
            wq_sb, wk_sb, wv_sb = w_sb

            # persistent activations
            qk_a = const.tile([128, S], BF16)   # [Q^T ; K^T]
            qk_b = const.tile([128, S], BF16)   # [K^T ; Q^T]  (partition swap)
            vt_sb = const.tile([64, S], BF16)   # V^T
            vnat_sb = const.tile([128, ST, H + 1], BF16)  # [V | ones] per k-tile
            osb = const.tile([H + 1, S], F32)    # O^T staging (SBUF)
            o_out = const.tile([128, ST, H], F32)  # final output staging

            # ---------- load x (cast f32->bf16) and build x^T ----------
            with tc.tile_pool(name="xload", bufs=1) as xload:
                x_bf = xload.tile([128, ST, D], BF16)
                x_src = x_dram.rearrange("(a p) d -> p a d", p=128)
                n_load = 8 if ST % 8 == 0 else 1
                step = ST // n_load
                for c in range(n_load):
                    nc.gpsimd.dma_start(
                        out=x_bf[:, c * step:(c + 1) * step, :],
                        in_=x_src[:, c * step:(c + 1) * step, :],
                    )

                # xt[p, j, s] = x[s, j*128 + p]  (== x^T[j*128+p, s])
                xt = xload.tile([128, DC, S], BF16)
                for si in range(ST):
                    eng = nc.sync if si % 2 == 0 else nc.scalar
                    eng.dma_start(
                        out=xt[:, :, si * 128:(si + 1) * 128],
                        in_=x_bf[:, si, :],
                        transpose=True,
                    )

                # ---------- projections ----------
                # col-packed Q (array cols 0-63) + K (cols 64-127):
                # psum rows 0-63 = Q^T chunk, rows 64-127 = K^T chunk
                with tc.tile_pool(name="proj_ps", bufs=2, space="PSUM") as proj_ps:
                    for qc in range(NQ):
                        pp = proj_ps.tile([128, QW], F32, name="pp_qk", tag="pp_qk")
                        for j in range(DC):
                            rhs = xt[:, j, qc * QW:(qc + 1) * QW]
                            nc.tensor.matmul(
                                pp[0:64, :], lhsT=wq_sb[:, j, :], rhs=rhs,
                                start=(j == 0), stop=(j == DC - 1),
                            )
                            nc.tensor.matmul(
                                pp[64:128, :], lhsT=wk_sb[:, j, :], rhs=rhs,
                                start=(j == 0), stop=(j == DC - 1),
                            )
                        nc.vector.tensor_copy(qk_a[:, qc * QW:(qc + 1) * QW], pp)
                    for qc in range(NQ):
                        pv = proj_ps.tile([64, QW], F32, name="pp_v", tag="pp_v")
                        for j in range(DC):
                            nc.tensor.matmul(
                                pv, lhsT=wv_sb[:, j, :],
                                rhs=xt[:, j, qc * QW:(qc + 1) * QW],
                                start=(j == 0), stop=(j == DC - 1),
                            )
                        nc.vector.tensor_copy(vt_sb[:, qc * QW:(qc + 1) * QW], pv)

                # partition-swapped duplicate: qk_b = [K^T ; Q^T]
                nc.sync.dma_start(out=qk_b[0:64, :], in_=qk_a[64:128, :])
                nc.scalar.dma_start(out=qk_b[64:128, :], in_=qk_a[0:64, :])

                # ---------- V natural (+ ones column) ----------
                with tc.tile_pool(name="v_ps", bufs=2, space="PSUM") as v_ps:
                    for st in range(ST):
                        vp = v_ps.tile([128, H], F32)
                        nc.tensor.matmul(
                            vp,
                            lhsT=vt_sb[:, st * 128:(st + 1) * 128],
                            rhs=ident_h,
                            start=True,
                            stop=True,
                        )
                        nc.vector.tensor_copy(vnat_sb[:, st, 0:H], vp)
                nc.vector.memset(vnat_sb[:, :, H:H + 1], 1.0)

            # ---------- main attention loop over k-tile pairs ----------
            # row-packed scores: even k-tile on PE rows 0-63 (operands from
            # qk_b lo / qk_a lo), odd k-tile on rows 64-127 (qk_a hi /
            # qk_b hi).  Each k-tile's scoresT psum is [128, 1024] (2 banks)
            # so exp runs in <=2 wide ops per k-tile.
            with (
                tc.tile_pool(name="attn_pool", bufs=3) as attn_pool,
                tc.tile_pool(name="s_ps", bufs=2, space="PSUM") as s_ps,
                tc.tile_pool(name="o_ps", bufs=1, space="PSUM") as o_ps,
            ):
                opsum = [
                    o_ps.tile([H + 1, QW], F32, name=f"opsum_{qc}", tag=f"opsum_{qc}")
                    for qc in range(NQ)
                ]

                def scores_mm(kt, qc, dst):
                    # dst: [128, 512] psum slice for this (kt, qc)
                    col = slice(kt * 128, (kt + 1) * 128)
                    qsl = slice(qc * QW, (qc + 1) * QW)
                    if kt % 2 == 0:
                        nc.tensor.matmul(
                            dst, lhsT=qk_b[0:64, col], rhs=qk_a[0:64, qsl],
                            start=True, stop=True,
                        )
                    else:
                        nc.tensor.matmul(
                            dst, lhsT=qk_a[64:128, col], rhs=qk_b[64:128, qsl],
                            start=True, stop=True,
                        )

                for tp in range((ST + 1) // 2):
                    kts = [2 * tp, 2 * tp + 1]
                    kts = [k for k in kts if k < ST]
                    sps = {}
                    for kt in kts:
                        sps[kt] = s_ps.tile(
                            [128, 2 * QW], F32, name="sps", tag="sps"
                        )
                    # interleave the two k-tiles' matmuls (row groups differ
                    # -> they run concurrently in the PE array)
                    for half in range(NQ // 2):
                        for sub in range(2):
                            qc = 2 * half + sub
                            for kt in kts:
                                if qc < (kt * 128) // QW:
                                    continue
                                scores_mm(
                                    kt, qc,
                                    sps[kt][:, (qc % 2) * QW:(qc % 2 + 1) * QW],
                                )
                    for kt in kts:
                        qc0 = (kt * 128) // QW
                        m = kt - qc0 * KPQ
                        attn = attn_pool.tile([128, S], BF16, name="attn", tag="attn")
                        v0 = kt * 128  # first valid q column
                        for half in range(NQ // 2):
                            hlo, hhi = half * 2 * QW, (half + 1) * 2 * QW
                            if hhi <= v0:
                                continue
                            lo = max(hlo, v0)
                            nc.scalar.activation(
                                out=attn[:, lo:hhi],
                                in_=sps[kt][:, lo - hlo:2 * QW],
                                func=mybir.ActivationFunctionType.Exp,
                                scale=inv_sqrt_h,
                            )
                        if m > 0:
                            nc.vector.memset(
                                attn[:, qc0 * QW:qc0 * QW + m * 128], 0.0
                            )
                        # triangular mask on the diagonal block
                        nc.vector.tensor_mul(
                            attn[:, kt * 128:(kt + 1) * 128],
                            attn[:, kt * 128:(kt + 1) * 128],
                            trimask,
                        )
                        for qc in range(qc0, NQ):
                            nc.tensor.matmul(
                                opsum[qc],
                                lhsT=vnat_sb[:, kt, :],
                                rhs=attn[:, qc * QW:(qc + 1) * QW],
                                start=(kt == 0),
                                stop=(kt == (qc + 1) * KPQ - 1),
                            )

                # ---------- O^T psum -> SBUF ----------
                for qc in range(NQ):
                    nc.vector.tensor_copy(osb[:, qc * QW:(qc + 1) * QW], opsum[qc])

            # ---------- finalize: transpose O^T -> O, normalize ----------
            with (
                tc.tile_pool(name="fin_ps", bufs=2, space="PSUM") as fin_ps,
                tc.tile_pool(name="fin_sb", bufs=2) as fin_sb,
            ):
                for t in range(ST):
                    op = fin_ps.tile([128, H + 1], F32)
                    nc.tensor.matmul(
                        op,
                        lhsT=osb[:, t * 128:(t + 1) * 128],
                        rhs=ident_o,
                        start=True,
                        stop=True,
                    )
                    recip = fin_sb.tile([128, 1], F32)
                    nc.vector.reciprocal(recip, op[:, H:H + 1])
                    nc.vector.tensor_scalar_mul(o_out[:, t, :], op[:, 0:H], recip)

            nc.sync.dma_start(
                out=out_dram.rearrange("(t p) h -> p t h", p=128),
                in_=o_out,
            )

    nc.compile()
    return nc


_NC_CACHE = {}


def _get_nc(S=SEQ, D=D_EMBED):
    key = (S, D)
    if key not in _NC_CACHE:
        _NC_CACHE[key] = build_attention_nc(S, D)
    return _NC_CACHE[key]


def kernel(x, Wq, Wk, Wv):
    """Full-input entry point: x [8, 2048, 1024] f32 -> [8, 2048, 64] f32."""
    from concourse.bass_utils import run_bass_kernel_spmd

    x = np.asarray(x, dtype=np.float32)
    Wq = np.ascontiguousarray(np.asarray(Wq, dtype=np.float32))
    Wk = np.ascontiguousarray(np.asarray(Wk, dtype=np.float32))
    Wv = np.ascontiguousarray(np.asarray(Wv, dtype=np.float32))
    assert x.shape == (BATCH, SEQ, D_EMBED), x.shape

    nc = _get_nc()
    in_maps = [
        {"x": np.ascontiguousarray(x[b]), "Wq": Wq, "Wk": Wk, "Wv": Wv}
        for b in range(BATCH)
    ]
    res = run_bass_kernel_spmd(nc, in_maps, core_ids=list(range(N_CORES)))
    return np.stack([res.results[b]["out"] for b in range(BATCH)], axis=0)


# revision 15
# speedup vs baseline: 482.9363x; 2.0745x over previous
"""Trainium2 Bass kernel for a single causal attention head.

Problem: x [8, 2048, 1024] f32, Wq/Wk/Wv [1024, 64] f32.
out[b] = softmax(causal(x[b] Wq (x[b] Wk)^T) / 8) @ (x[b] Wv)   -> [8, 2048, 64] f32

Sharding: data-parallel over batch. Each of the 8 NeuronCores runs the same
single-core program on its own batch element (no collectives).

Per-core dataflow (matmuls in bf16, fp32 PSUM accumulation):
  1. SWDGE cast-DMA x f32->bf16 into SBUF (8 chunks).
  2. x^T via PE matmul-by-identity (xbar DMA-transpose serializes against
     all other DMA traffic on HW, so PE does it and overlaps the loads).
     Emission is interleaved per q-chunk group (transposes -> projections ->
     V-natural) so the PE stream flows as load chunks land.
  3. Projections: col-packed matmuls give qk_a = [Q^T ; K^T] on 128
     partitions; partition-swap DMAs build qk_b = [K^T ; Q^T] so scores can
     be row-packed (two k-tiles concurrently, contraction h=64 each).
  4. scoresT[k,q] = K^T[:,kt].T @ Q^T, exp on ScalarE (1/sqrt(64) folded
     in), causal handling by loop bounds + memset + one triangular-mask
     multiply per diagonal block.
  5. attnT @ V with lhsT = [V | ones]: PSUM row 64 accumulates the softmax
     denominators alongside the 64 output dims: O^T[65, q].
  6. O = (O^T).T via one xbar transpose (bf16), DVE reciprocal +
     tensor_scalar per s-tile, DMA out f32.
"""

import contextlib
import math
import sys

import numpy as np

if "/opt/trn_rl_repo" not in sys.path:
    sys.path.insert(0, "/opt/trn_rl_repo")

import concourse.bacc as bacc
import concourse.tile as tile
from concourse import mybir
from concourse.masks import make_identity

BATCH = 8
SEQ = 2048
D_EMBED = 1024
HEAD = 64
N_CORES = 8

F32 = mybir.dt.float32
BF16 = mybir.dt.bfloat16


def build_attention_nc(S=SEQ, D=D_EMBED, repeat=1, phase="full", qk_dup="dma",
                       xpose="pe"):
    """Build the single-core Bass program for one batch element.

    repeat > 1 wraps the body in a hardware For_i loop (for timing).
    phase ablations: "load" | "transpose" | "frontend" | "full".
    """
    H = HEAD
    ST = S // 128          # s-tiles == k-tiles
    DC = D // 128          # d-chunks
    QW = 512               # q-chunk width
    NQ = S // QW           # q-chunks
    KPQ = QW // 128        # k-tiles per q-chunk
    SPQ = ST // NQ         # s-tiles per q-chunk (= KPQ)
    inv_sqrt_h = 1.0 / math.sqrt(H)

    nc = bacc.Bacc("TRN2", target_bir_lowering=False, debug=False)

    x_dram = nc.dram_tensor("x", [S, D], F32, kind="ExternalInput").ap()
    wq_dram = nc.dram_tensor("Wq", [D, H], F32, kind="ExternalInput").ap()
    wk_dram = nc.dram_tensor("Wk", [D, H], F32, kind="ExternalInput").ap()
    wv_dram = nc.dram_tensor("Wv", [D, H], F32, kind="ExternalInput").ap()
    out_dram = nc.dram_tensor("out", [S, H], F32, kind="ExternalOutput").ap()

    with tile.TileContext(nc) as tc:
      with (tc.For_i(0, repeat, 1) if repeat > 1 else contextlib.nullcontext()):
        with tc.tile_pool(name="const", bufs=1) as const:
            ident_128 = const.tile([128, 128], BF16)
            make_identity(nc, ident_128)
            # trimask[k_local, q_local] = 1.0 if q_local >= k_local else 0.0
            trimask = const.tile([128, 128], BF16)
            nc.gpsimd.memset(trimask, 1.0)
            nc.gpsimd.affine_select(
                out=trimask,
                in_=trimask,
                compare_op=mybir.AluOpType.is_ge,
                fill=0.0,
                base=0,
                pattern=[[1, 128]],
                channel_multiplier=-1,
            )

            # weights -> bf16 [128, DC, H]; w_sb[p, j, h] = W[j*128+p, h]
            w_sb = []
            for name, wd in (("wq", wq_dram), ("wk", wk_dram), ("wv", wv_dram)):
                w_t = const.tile([128, DC, H], BF16, name=f"w_{name}", tag=f"w_{name}")
                nc.gpsimd.dma_start(out=w_t, in_=wd.rearrange("(j p) h -> p j h", p=128))
                w_sb.append(w_t)
            wq_sb, wk_sb, wv_sb = w_sb

            # persistent activations
            qk_a = const.tile([128, S], BF16)   # [Q^T ; K^T]
            qk_b = const.tile([128, S], BF16)   # [K^T ; Q^T]  (partition swap)
            vt_sb = const.tile([64, S], BF16)   # V^T
            vnat_sb = const.tile([128, ST, H + 1], BF16)  # [V | ones] per k-tile
            osb = const.tile([80, S], BF16)      # O^T staging (bf16, padded)
            onat = const.tile([128, ST, 80], BF16)  # transposed O | sums
            o_out = const.tile([128, ST, H], F32)   # final output staging

            # ---------- load x (cast f32->bf16) ----------
            with tc.tile_pool(name="xload", bufs=1) as xload:
                x_bf = xload.tile([128, ST, D], BF16)
                x_src = x_dram.rearrange("(a p) d -> p a d", p=128)
                n_load = 2 * NQ if ST == 2 * NQ * (ST // (2 * NQ)) else 1
                step = ST // n_load
                for c in range(n_load):
                    nc.gpsimd.dma_start(
                        out=x_bf[:, c * step:(c + 1) * step, :],
                        in_=x_src[:, c * step:(c + 1) * step, :],
                    )

                # xt2[p, g, c] = x[(g//DC)*128 + c, (g%DC)*128 + p]
                xt2 = xload.tile([128, ST * DC, 128], BF16)

                def xt_rhs(j, qc):
                    # [128, SPQ, 128] AP: d-chunk j, q-chunk qc
                    return xt2[:, qc * SPQ * DC + j:(qc + 1) * SPQ * DC:DC, :]

                def emit_xpose_pe(si, xp_ps):
                    # x^T for s-tile si: out = x_block.T @ I, 4 d-blocks per
                    # PSUM bank, one wide copy-cast per bank
                    for jg in range(DC // 4):
                        xp = xp_ps.tile([128, 512], F32, name="xp", tag="xp")
                        for k in range(4):
                            j = jg * 4 + k
                            nc.tensor.matmul(
                                xp[:, k * 128:(k + 1) * 128],
                                lhsT=x_bf[:, si, j * 128:(j + 1) * 128],
                                rhs=ident_128,
                                start=True, stop=True,
                            )
                        dst = xt2[:, si * DC + jg * 4:si * DC + jg * 4 + 4, :]
                        if (si + jg) % 2 == 0:
                            nc.vector.tensor_copy(dst, xp)
                        else:
                            nc.scalar.copy(dst, xp)

                if phase == "load":
                    for t in range(ST):
                        nc.vector.tensor_copy(
                            o_out[:, t, :],
                            x_bf.rearrange("p a b -> p (a b)")[:, t * H:(t + 1) * H],
                        )
                    nc.sync.dma_start(
                        out=out_dram.rearrange("(t p) h -> p t h", p=128),
                        in_=o_out,
                    )

                elif xpose == "xbar" and phase != "load":
                    for c in range(n_load):
                        eng = nc.sync if c % 2 == 0 else nc.scalar
                        eng.dma_start(
                            out=xt2[:, c * step * DC:(c + 1) * step * DC, :],
                            in_=x_bf[:, c * step:(c + 1) * step, :]
                                .rearrange("p a d -> p (a d)"),
                            transpose=True,
                        )

                # ---------- interleaved: transpose -> proj -> vnat per qc ----
                if phase != "load":
                    with (
                        tc.tile_pool(name="xp_ps", bufs=2, space="PSUM") as xp_ps,
                        tc.tile_pool(name="proj_ps", bufs=2, space="PSUM") as proj_ps,
                        tc.tile_pool(name="v_ps", bufs=2, space="PSUM") as v_ps,
                    ):
                        for qc in range(NQ):
                            if xpose == "pe":
                                for si in range(qc * SPQ, (qc + 1) * SPQ):
                                    emit_xpose_pe(si, xp_ps)
                            if phase == "transpose":
                                continue
                            qsl = slice(qc * QW, (qc + 1) * QW)
                            # col-packed Q (cols 0-63) + K (cols 64-127)
                            pp = proj_ps.tile([128, QW], F32, name="pp_qk", tag="pp_qk")
                            for j in range(DC):
                                rhs = xt_rhs(j, qc)
                                nc.tensor.matmul(
                                    pp[0:64, :], lhsT=wq_sb[:, j, :], rhs=rhs,
                                    start=(j == 0), stop=(j == DC - 1),
                                    skip_group_check=True,
                                )
                                nc.tensor.matmul(
                                    pp[64:128, :], lhsT=wk_sb[:, j, :], rhs=rhs,
                                    start=(j == 0), stop=(j == DC - 1),
                                    skip_group_check=True,
                                )
                            nc.vector.tensor_copy(qk_a[:, qsl], pp)
                            if qk_dup == "dma":
                                nc.sync.dma_start(
                                    out=qk_b[0:64, qsl], in_=qk_a[64:128, qsl]
                                )
                                nc.scalar.dma_start(
                                    out=qk_b[64:128, qsl], in_=qk_a[0:64, qsl]
                                )
                            else:  # "mm": second col-packed pass [K^T ; Q^T]
                                pb = proj_ps.tile(
                                    [128, QW], F32, name="pp_qk2", tag="pp_qk"
                                )
                                for j in range(DC):
                                    rhs = xt_rhs(j, qc)
                                    nc.tensor.matmul(
                                        pb[0:64, :], lhsT=wk_sb[:, j, :], rhs=rhs,
                                        start=(j == 0), stop=(j == DC - 1),
                                        skip_group_check=True,
                                    )
                                    nc.tensor.matmul(
                                        pb[64:128, :], lhsT=wq_sb[:, j, :], rhs=rhs,
                                        start=(j == 0), stop=(j == DC - 1),
                                        skip_group_check=True,
                                    )
                                nc.vector.tensor_copy(qk_b[:, qsl], pb)
                            # V^T for this chunk
                            pv = proj_ps.tile([64, QW], F32, name="pp_v", tag="pp_v")
                            for j in range(DC):
                                nc.tensor.matmul(
                                    pv, lhsT=wv_sb[:, j, :], rhs=xt_rhs(j, qc),
                                    start=(j == 0), stop=(j == DC - 1),
                                )
                            nc.scalar.copy(vt_sb[:, qsl], pv)
                            # V natural (+ ones col) for this chunk's s-tiles
                            for st in range(qc * SPQ, (qc + 1) * SPQ):
                                vp = v_ps.tile([128, H], F32, name="vp", tag="vp")
                                nc.tensor.matmul(
                                    vp,
                                    lhsT=vt_sb[:, st * 128:(st + 1) * 128],
                                    rhs=ident_128[0:64, 0:64],
                                    start=True, stop=True,
                                )
                                nc.scalar.copy(vnat_sb[:, st, 0:H], vp)
                        if phase != "transpose":
                            nc.vector.memset(vnat_sb[:, :, H:H + 1], 1.0)

                if phase == "transpose":
                    for t in range(ST):
                        nc.vector.tensor_copy(
                            o_out[:, t, :],
                            xt2.rearrange("p a b -> p (a b)")[:, t * H:(t + 1) * H],
                        )
                    nc.sync.dma_start(
                        out=out_dram.rearrange("(t p) h -> p t h", p=128),
                        in_=o_out,
                    )
                elif phase == "frontend":
                    for t in range(ST):
                        nc.vector.tensor_copy(
                            o_out[:, t, :], qk_a[:, t * 128:t * 128 + H]
                        )
                    nc.vector.tensor_copy(o_out[:, 0, :], vnat_sb[:, 0, 0:H])
                    nc.sync.dma_start(
                        out=out_dram.rearrange("(t p) h -> p t h", p=128),
                        in_=o_out,
                    )

            if phase == "full":
                # ------- main attention loop over k-tile pairs -------
                # row-packed scores: even k-tile on PE rows 0-63 (qk_b lo /
                # qk_a lo), odd k-tile on rows 64-127 (qk_a hi / qk_b hi).
                # Each k-tile's scoresT psum is [128, 1024] (2 banks) so exp
                # runs in <=2 wide ops per k-tile.
                with (
                    tc.tile_pool(name="attn_pool", bufs=3) as attn_pool,
                    tc.tile_pool(name="s_ps", bufs=1, space="PSUM") as s_ps,
                    tc.tile_pool(name="o_ps", bufs=1, space="PSUM") as o_ps,
                ):
                    opsum = [
                        o_ps.tile([H + 1, QW], F32,
                                  name=f"opsum_{qc}", tag=f"opsum_{qc}")
                        for qc in range(NQ)
                    ]

                    def scores_mm(kt, qc, dst):
                        col = slice(kt * 128, (kt + 1) * 128)
                        qsl = slice(qc * QW, (qc + 1) * QW)
                        if kt % 2 == 0:
                            nc.tensor.matmul(
                                dst, lhsT=qk_b[0:64, col], rhs=qk_a[0:64, qsl],
                                start=True, stop=True,
                            )
                        else:
                            nc.tensor.matmul(
                                dst, lhsT=qk_a[64:128, col], rhs=qk_b[64:128, qsl],
                                start=True, stop=True,
                            )

                    n_half = (NQ + 1) // 2
                    for tp in range((ST + 1) // 2):
                        kts = [k for k in (2 * tp, 2 * tp + 1) if k < ST]
                        attn = {
                            kt: attn_pool.tile([128, S], BF16,
                                               name="attn", tag=f"attn{kt % 2}")
                            for kt in kts
                        }
                        for half in range(n_half):
                            hlo = half * 2 * QW
                            hhi = min(hlo + 2 * QW, S)
                            live = [kt for kt in kts if hhi > kt * 128]
                            sps = {
                                kt: s_ps.tile([128, 2 * QW], F32,
                                              name="sps", tag=f"sps{kt % 2}")
                                for kt in live
                            }
                            for sub in range(2):
                                qc = 2 * half + sub
                                if qc >= NQ:
                                    continue
                                for kt in live:
                                    if qc < (kt * 128) // QW:
                                        continue
                                    scores_mm(
                                        kt, qc,
                                        sps[kt][:, sub * QW:(sub + 1) * QW],
                                    )
                            for kt in live:
                                lo = max(hlo, kt * 128)
                                nc.scalar.activation(
                                    out=attn[kt][:, lo:hhi],
                                    in_=sps[kt][:, lo - hlo:hhi - hlo],
                                    func=mybir.ActivationFunctionType.Exp,
                                    scale=inv_sqrt_h,
                                )
                        for kt in kts:
                            qc0 = (kt * 128) // QW
                            m = kt - qc0 * KPQ
                            if m > 0:
                                nc.vector.memset(
                                    attn[kt][:, qc0 * QW:qc0 * QW + m * 128], 0.0
                                )
                            nc.vector.tensor_mul(
                                attn[kt][:, kt * 128:(kt + 1) * 128],
                                attn[kt][:, kt * 128:(kt + 1) * 128],
                                trimask,
                            )
                            for qc in range(qc0, NQ):
                                nc.tensor.matmul(
                                    opsum[qc],
                                    lhsT=vnat_sb[:, kt, :],
                                    rhs=attn[kt][:, qc * QW:(qc + 1) * QW],
                                    start=(kt == 0),
                                    stop=(kt == (qc + 1) * KPQ - 1),
                                )

                    # ------- O^T psum -> SBUF (bf16) -------
                    nc.vector.memset(osb[64:80, :], 0.0)
                    for qc in range(NQ):
                        nc.vector.tensor_copy(
                            osb[0:H + 1, qc * QW:(qc + 1) * QW], opsum[qc]
                        )

                # ------- finalize: xbar-transpose O^T -> O, normalize -------
                with tc.tile_pool(name="fin_sb", bufs=4) as fin_sb:
                    nc.sync.dma_start(out=onat, in_=osb, transpose=True)
                    for t in range(ST):
                        recip = fin_sb.tile([128, 1], F32)
                        nc.vector.reciprocal(recip, onat[:, t, H:H + 1])
                        nc.vector.tensor_scalar_mul(
                            o_out[:, t, :], onat[:, t, 0:H], recip
                        )

                nc.sync.dma_start(
                    out=out_dram.rearrange("(t p) h -> p t h", p=128),
                    in_=o_out,
                )

    nc.compile()
    return nc


_NC_CACHE = {}


def _get_nc(S=SEQ, D=D_EMBED):
    key = (S, D)
    if key not in _NC_CACHE:
        _NC_CACHE[key] = build_attention_nc(S, D)
    return _NC_CACHE[key]


def kernel(x, Wq, Wk, Wv):
    """Full-input entry point: x [8, 2048, 1024] f32 -> [8, 2048, 64] f32."""
    from concourse.bass_utils import run_bass_kernel_spmd

    x = np.asarray(x, dtype=np.float32)
    Wq = np.ascontiguousarray(np.asarray(Wq, dtype=np.float32))
    Wk = np.ascontiguousarray(np.asarray(Wk, dtype=np.float32))
    Wv = np.ascontiguousarray(np.asarray(Wv, dtype=np.float32))
    assert x.shape == (BATCH, SEQ, D_EMBED), x.shape

    nc = _get_nc()
    in_maps = [
        {"x": np.ascontiguousarray(x[b]), "Wq": Wq, "Wk": Wk, "Wv": Wv}
        for b in range(BATCH)
    ]
    res = run_bass_kernel_spmd(nc, in_maps, core_ids=list(range(N_CORES)))
    return np.stack([res.results[b]["out"] for b in range(BATCH)], axis=0)
